# revision 8
# baseline (speedup 1.0000x reference)
"""DQS encoder (dual-quaternion skinning blend) Trainium2 kernel, v2.

Contract: kernel(x, weights, VR) -> (8_000_000,) float32, matching
reference._dqs numerics. Data-parallel over nodes across 8 NeuronCores.

v2 strategy (vs the v1 strip-transpose pipeline): move the weight
transpose to the HOST and make the super-window (86,016 nodes) the
partition-chunking unit. Per core the 250k nodes are covered by 3
super-windows (last one overlaps, idempotent writes). Within a
super-window, partition p owns nodes [base + 672p, base + 672p + 672),
as 112 slabs of 6 nodes.

Host packs weights into a fp16 strip stream
  w_pk[r, (g*112 + B)*128 + p] = W[base_g + 672p + 6B + nu, k],
r = 20*nu + k, so every kernel DMA is fully contiguous per partition:

  1. W DMA per super-window: [120, 14336] fp16, 120 x 28KB descriptors.
  2. Per slab B: one matmul, stationary = strip slab [120, 128]
     (cols = p), moving = host-built block-diag bd [120, 32]:
     out[p, 6c+nu] = q_c(node 672p + 6B + nu). 16 slabs fill one PSUM
     bank [128, 512]; 7 banks' worth per super-window.
  3. One ACT copy PSUM -> t_sb per 16-slab chunk.
  4. VR DMA per super-window: [128, 2688] f32 contiguous
     (vrt[p, 4d + comp] = VR comp of node base + 672p + d, d = 6B + nu
     -- exactly matching the q layout t_sb[p, 32B + 6c + nu]).
  5. DVE math, cross-product form of the rotation (identical algebra
     to reference's G @ (H^T v), division by |q|^2 instead of
     normalize):
       t  = u x v + d*v
       y  = v + (1/|q|^2) * 2*(u x t)
     y written in-place over v slots of the VR tile; radius untouched.
  6. One contiguous DMA of the VR tile back out.

fp16 weights halve W HBM traffic (10MB/core/repeat); VR/out stay f32.
Per-core HBM traffic per repeat ~18MB -> ~50us roofline at 358GB/s.

Walrus's codegen accepts only ONE sync-wait command on most encodings
(incl. loop-barrier NoOps and Drains). Countermeasures: all SWDGE
completion semaphores are collapsed to one lane, and traced nop
placeholders are inserted before phase-leader instructions; a
post-schedule pass moves any excess waits onto the placeholders
(same-engine program order then enforces the dependency).

repeats can run as a hardware For_i loop (hw_loop=True): constant
program size for any repeat count, enabling honest repeat-slope timing.
"""

import sys
from contextlib import ExitStack

import numpy as np

sys.path.insert(0, "/opt/trn_rl_repo")

import concourse.bass as bass  # noqa: E402
import concourse.tile as tile  # noqa: E402
from concourse import mybir  # noqa: E402
from concourse import tile_sem_assignment as _tsa  # noqa: E402
from concourse.bass_utils import run_bass_kernel_spmd  # noqa: E402

# One SWDGE completion lane: DMA ticks subsume each other (the SWDGE queue
# is FIFO), so no instruction ever needs two DMA waits.
_tsa.NUM_SWDGE_GLOBAL_SEMS = 1

FP = mybir.dt.float32
BF = mybir.dt.float16
OP = mybir.AluOpType

N_NODES = 2_000_000
N_CORES = 8
NPC = N_NODES // N_CORES  # 250_000 nodes per core

G_NODES = 6          # nodes per strip column
KW = 20              # weights per node
F = G_NODES * KW     # 120 strip rows
CHUNK = 16           # slabs per PSUM bank fill (16 * 32 cols = 512)
N_CHUNK = 7          # PSUM-bank fills per super-window
SLABS = CHUNK * N_CHUNK          # 112 slabs per super-window
NPP = G_NODES * SLABS            # 672 nodes per partition
GWIN = 128 * NPP                 # 86_016 nodes per super-window


def _window_bases(npc):
    n_full = npc // GWIN
    bases = [w * GWIN for w in range(n_full)]
    if npc - n_full * GWIN > 0:
        bases.append(npc - GWIN)  # overlapped tail window (idempotent writes)
    return bases


def _fview(ap, off, dims):
    """Strided free-dim view of a 2-D SBUF/PSUM AP. dims = [[step, count],...]."""
    return bass.AP(tensor=ap.tensor, offset=ap.offset + off, ap=[ap.ap[0]] + dims)


class _Ph:
    """Pool of traced carrier nops for the excess-wait retarget pass."""

    def __init__(self):
        self.names = set()

    def seed(self, nc, pool, n_per_engine=96):
        sem = nc.alloc_semaphore("ph_carrier_dummy")
        for ns in (nc.tensor, nc.gpsimd, nc.scalar, nc.sync):
            for _ in range(n_per_engine):
                p = ns.wait_ge(sem, 0).ins
                self.names.add(p.name)
        junk = pool.tile([1, n_per_engine], mybir.dt.float32, tag="phjunk")
        for k in range(n_per_engine):
            p = nc.vector.memset(junk[0:1, k : k + 1], 0.0).ins
            self.names.add(p.name)


def _retarget_waits(nc, ph_names):
    """Relocate seeded carrier nops to absorb excess sync-waits (walrus
    accepts only one sync-wait on most instruction encodings)."""
    import bass_rust

    moved = 0
    skip = ("InstEventSemaphore",)
    allow = ("InstMatmult", "InstActivation", "InstDMACopy", "InstDrain",
             "InstTensorTensor", "InstTensorScalarPtr", "InstTensorReduce",
             "InstReciprocal", "InstCopy", "InstTensorCopy", "InstNoOp",
             "InstMemset")
    blocks = list(nc.main_func.blocks)
    pool = {}
    plan = {}
    consumed = set()
    for bb in blocks:
        for ins in bb.instructions:
            if ins.name in ph_names and (
                ins.sync_info is None or not ins.sync_info.on_update
            ):
                pool.setdefault(ins.engine, []).append(ins)
    for bb in blocks:
        for ins in bb.instructions:
            if ins.name in ph_names:
                continue
            if type(ins).__name__ in skip or type(ins).__name__ not in allow:
                continue
            si = ins.sync_info
            if si is not None and len(si.on_wait) > 1:
                waits = list(si.on_wait)
                excess = waits[:-1]
                phs = pool.get(ins.engine, [])
                carriers = []
                for w in excess:
                    if phs:
                        p = phs.pop()
                    else:
                        # Synthesize a fresh carrier nop (no seeded carrier
                        # left on this engine, e.g. DVE where every traced
                        # op acquires a clock update).
                        p = mybir.InstNoOp(
                            name=nc.get_next_instruction_name(),
                            engine=ins.engine, ins=[], outs=[],
                        )
                        ph_names.add(p.name)
                    p.sync_info = bass_rust.SyncInfo(on_wait=[w], on_update=[])
                    try:
                        p.bass_scheduled_tick = ins.bass_scheduled_tick
                    except Exception:
                        pass
                    consumed.add(p.name)
                    carriers.append(p)
                    moved += 1
                ins.sync_info = bass_rust.SyncInfo(
                    on_wait=waits[-1:], on_update=list(si.on_update)
                )
                plan[ins.name] = carriers
    unused = set()
    for phs in pool.values():
        unused.update(p.name for p in phs)
    for bb in blocks:
        out = []
        for ins in bb.instructions:
            if ins.name in consumed or ins.name in unused:
                continue
            out.extend(plan.get(ins.name, ()))
            out.append(ins)
        bb.instructions = out
    return moved


def _fix_loop_swdge_reset(nc):
    """Walrus's codegen cannot encode InstIncSwdgeSem ("ISA wrong
    length"), which Tile's For_i reset block uses to rewind the SWDGE
    completion semaphore between iterations. Replace it with a Pool NoOp
    carrying a sem-sub-imm update of the per-iteration SWDGE total (the
    same mechanism Tile uses for the engine-clock resets). All body DMAs
    are complete at the reset point (the reset block is behind a drain +
    all-engine barrier), so the value is statically exact."""
    import bass_rust

    blocks = list(nc.main_func.blocks)
    body = [bb for bb in blocks if bb.name.endswith("_body")]
    if not body:
        return 0
    # per-sem SWDGE increment totals inside the loop body
    totals = {}
    names = {}
    for bb in body:
        for ins in bb.instructions:
            si = ins.sync_info
            if type(ins).__name__ == "InstDMACopy" and si is not None:
                for u in si.on_update:
                    if u.ant_name.startswith("DMASW"):
                        totals[u.id] = totals.get(u.id, 0) + u.update_value
                        names[u.id] = u.ant_name
    replaced = 0
    for bb in blocks:
        out = []
        for ins in bb.instructions:
            if type(ins).__name__ != "InstIncSwdgeSem":
                out.append(ins)
                continue
            si = ins.sync_info
            waits = list(si.on_wait) if si else []
            if bb.name.endswith("_reset"):
                ups = [
                    bass_rust.SyncUpdate(
                        sync_type="semaphore", id=i, ant_name=names[i],
                        update_mode="sem-sub-imm", update_value=v,
                        update_reg=None,
                    )
                    for i, v in sorted(totals.items()) if v > 0
                ]
            else:
                # skip-block instance: loop trip count is always >= 2 in
                # hw_loop mode, the skip path never executes; keep only
                # its wait so the block stays well-formed.
                ups = []
            rep = mybir.InstNoOp(
                name=nc.get_next_instruction_name(),
                engine=ins.engine, ins=[], outs=[],
            )
            rep.sync_info = bass_rust.SyncInfo(
                on_wait=waits[:1], on_update=ups[:1]
            )
            out.append(rep)
            for extra in ups[1:]:
                rep2 = mybir.InstNoOp(
                    name=nc.get_next_instruction_name(),
                    engine=ins.engine, ins=[], outs=[],
                )
                rep2.sync_info = bass_rust.SyncInfo(on_wait=[], on_update=[extra])
                out.append(rep2)
            replaced += 1
        bb.instructions = out
    return replaced


def build_program(npc=NPC, repeats=1, hw_loop=False, split_waits=True,
                  mode="full"):
    # mode: "full" | "nomath" (skip DVE math; out = raw VR) |
    #       "dma" (only the three DMAs) — timing ablations, output wrong.
    nc = bass.Bass()

    bases = _window_bases(npc)
    nwin = len(bases)

    w_d = nc.dram_tensor("w", [F, nwin * SLABS * 128], BF, kind="ExternalInput")
    vr_d = nc.dram_tensor("vr", [npc * 4], FP, kind="ExternalInput")
    bd_d = nc.dram_tensor("bd", [F, 32], BF, kind="ExternalInput")
    out_d = nc.dram_tensor("out", [npc * 4], FP, kind="ExternalOutput")

    unroll = 1 if hw_loop else repeats

    ph = _Ph()

    with tile.TileContext(nc) as tc, ExitStack() as ctx:
        const = ctx.enter_context(tc.tile_pool(name="const", bufs=1))
        ph.seed(nc, const, n_per_engine=64 + 48 * unroll)
        strip_p = ctx.enter_context(tc.tile_pool(name="strip", bufs=2))
        grp_p = ctx.enter_context(tc.tile_pool(name="grp", bufs=2))
        scr_p = ctx.enter_context(tc.tile_pool(name="scr", bufs=1))
        sq_p = ctx.enter_context(tc.tile_pool(name="sq", bufs=2))
        qps_p = ctx.enter_context(tc.tile_pool(name="qps", bufs=4, space="PSUM"))

        bd_sb = const.tile([F, 32], BF)
        nc.sync.dma_start(out=bd_sb[:], in_=bd_d[:, :])
        s0_junk = const.tile([1, 16], FP)

        if hw_loop and repeats > 1:
            # Hardware repeat loop: constant program size for any repeat
            # count (same addresses each iteration, idempotent writes).
            ctx.enter_context(tc.For_i(0, repeats, 1))

        for _rep in range(unroll):
            for gw, base in enumerate(bases):
                # --- W strip load: one contiguous DMA per super-window ---
                strip = strip_p.tile([F, SLABS * 128], BF, tag="strip")
                nc.sync.dma_start(
                    out=strip[:],
                    in_=w_d[:, gw * SLABS * 128 : (gw + 1) * SLABS * 128],
                )

                # --- VR load: one contiguous DMA per super-window ---
                vrt = grp_p.tile([128, NPP * 4], FP, tag="vrt")
                nc.sync.dma_start(
                    out=vrt[:],
                    in_=vr_d[base * 4 : (base + GWIN) * 4].rearrange(
                        "(p f) -> p f", p=128
                    ),
                )

                if mode == "dma":
                    r0 = vrt[0:1, 3:4]
                    nc.vector.tensor_copy(r0, r0)  # WAR split vs prev out-DMA
                    nc.sync.dma_start(
                        out=out_d[base * 4 : (base + GWIN) * 4].rearrange(
                            "(p f) -> p f", p=128
                        ),
                        in_=vrt[:],
                    )
                    # keep the strip load live (read 1 elem so DCE keeps it)
                    nc.vector.tensor_copy(s0_junk[0:1, gw : gw + 1],
                                          strip[0:1, 0:1])
                    continue

                # --- blend matmuls + PSUM drain ---
                t_sb = grp_p.tile([128, N_CHUNK * 512], FP, tag="t_sb")
                for ch in range(N_CHUNK):
                    qps = qps_p.tile([128, 512], FP, tag="qps")
                    for b in range(CHUNK):
                        col = (ch * CHUNK + b) * 128
                        nc.tensor.matmul(
                            qps[:, 32 * b : 32 * (b + 1)],
                            strip[:, col : col + 128],
                            bd_sb[:],
                            start=True, stop=True, tile_position=(0, 0),
                        )
                    nc.scalar.copy(t_sb[:, 512 * ch : 512 * (ch + 1)], qps[:])

                if mode == "nomath":
                    nc.vector.tensor_copy(s0_junk[0:1, gw : gw + 1],
                                          t_sb[0:1, 0:1])
                    r0 = vrt[0:1, 3:4]
                    nc.vector.tensor_copy(r0, r0)
                    nc.sync.dma_start(
                        out=out_d[base * 4 : (base + GWIN) * 4].rearrange(
                            "(p f) -> p f", p=128
                        ),
                        in_=vrt[:],
                    )
                    continue

                # ---------- math over the super-window ----------
                a = SLABS       # merged slab dim
                fd = 6 * a      # nodes per partition (672)

                def qv(c):  # quat component plane view of t_sb
                    return _fview(t_sb[:], 6 * c, [[32, a], [1, 6]])

                def vv(c):  # VR component plane view
                    return _fview(vrt[:], c, [[24, a], [4, 6]])

                def sh(tl):  # scratch tile shaped to match views
                    return _fview(tl[:], 0, [[6, a], [1, 6]])

                A, Bq, C, D = qv(0), qv(1), qv(2), qv(3)
                v1, v2, v3 = vv(0), vv(1), vv(2)

                t1 = scr_p.tile([128, fd], FP, tag="t1")
                t2 = scr_p.tile([128, fd], FP, tag="t2")
                t3 = scr_p.tile([128, fd], FP, tag="t3")
                s1 = scr_p.tile([128, fd], FP, tag="s1")
                s2 = scr_p.tile([128, fd], FP, tag="s2")

                # Split the multi-producer dependency join of the first DVE
                # op across chained 1-element copies (walrus: one wait/op).
                # Dest s2 is clobbered later.
                for ch in range(N_CHUNK):
                    nc.vector.tensor_copy(
                        s2[0:1, ch : ch + 1],
                        t_sb[0:1, 512 * ch : 512 * ch + 1],
                    )
                nc.vector.tensor_copy(s2[0:1, 7:8], vrt[0:1, 0:1])

                def tt(out, a_, b_, op):
                    nc.vector.tensor_tensor(out, a_, b_, op)

                def stt(out, in0, scalar, in1):
                    nc.vector.scalar_tensor_tensor(
                        out=out, in0=in0, scalar=scalar, in1=in1,
                        op0=OP.mult, op1=OP.mult)

                w1 = scr_p.tile([128, fd], FP, tag="w1")
                w2 = scr_p.tile([128, fd], FP, tag="w2")
                w3 = scr_p.tile([128, fd], FP, tag="w3")
                n2 = scr_p.tile([128, fd], FP, tag="n2")
                inv = scr_p.tile([128, fd], FP, tag="inv")
                sq = sq_p.tile([128, 24 * a], FP, tag="sq")

                # t = u x v + d*v
                for tout, (f1, e1), (f2, e2), (f3, e3) in (
                    (t1, (Bq, v3), (C, v2), (D, v1)),
                    (t2, (C, v1), (A, v3), (D, v2)),
                    (t3, (A, v2), (Bq, v1), (D, v3)),
                ):
                    tt(sh(s1), f1, e1, OP.mult)
                    stt(sh(s2), f2, -1.0, e2)
                    tt(sh(s1), sh(s1), sh(s2), OP.add)
                    tt(sh(s2), f3, e3, OP.mult)
                    tt(sh(tout), sh(s1), sh(s2), OP.add)

                # w = 2*(u x t)
                for wout, (f1, e1), (f2, e2) in (
                    (w1, (Bq, t3), (C, t2)),
                    (w2, (C, t1), (A, t3)),
                    (w3, (A, t2), (Bq, t1)),
                ):
                    stt(sh(s1), f1, 2.0, sh(e1))
                    stt(sh(s2), f2, 2.0, sh(e2))
                    tt(sh(wout), sh(s1), sh(s2), OP.subtract)

                # n2 = sum of squares over the 4 quat components; inv = 1/n2
                sq_in = _fview(t_sb[:], 0, [[32, a], [1, 24]])
                sq_out = _fview(sq[:], 0, [[24, a], [1, 24]])
                nc.scalar.activation(
                    sq_out, sq_in, mybir.ActivationFunctionType.Square,
                )
                sqv = _fview(sq[:], 0, [[24, a], [1, 6], [6, 4]])
                nc.vector.tensor_reduce(
                    out=sh(n2), in_=sqv, axis=mybir.AxisListType.X, op=OP.add
                )
                nc.vector.reciprocal(out=inv[:, :fd], in_=n2[:, :fd])

                # WAR split: the y-passes write vrt in place and would
                # inherit waits on the previous occupant's out-DMA.
                r0 = vrt[0:1, 3:4]
                nc.vector.tensor_copy(r0, r0)

                # y_c = v_c + inv * w_c   (written in place over v_c)
                for wsrc, vdst_ in ((w1, v1), (w2, v2), (w3, v3)):
                    tt(sh(s1), sh(inv), sh(wsrc), OP.mult)
                    tt(vdst_, sh(s1), vdst_, OP.add)

                # --- store: one contiguous DMA per super-window ---
                nc.sync.dma_start(
                    out=out_d[base * 4 : (base + GWIN) * 4].rearrange(
                        "(p f) -> p f", p=128
                    ),
                    in_=vrt[:],
                )

    if split_waits:
        _retarget_waits(nc, ph.names)
    if hw_loop and repeats > 1:
        _fix_loop_swdge_reset(nc)
    return nc


def make_bd(x):
    """Host-side block-diag blend matrix (120, 32) bf16 from x (40,)."""
    qm4p1 = np.asarray(x, np.float32).reshape(10, 4)
    qm4p2 = np.zeros_like(qm4p1)
    qm4p2[:, 3] = 1.0
    qm4 = np.concatenate([qm4p1, qm4p2], axis=0)  # (20, 4)
    bd = np.zeros((F, 32), np.float32)
    for nu in range(G_NODES):
        for c in range(4):
            bd[KW * nu : KW * (nu + 1), 6 * c + nu] = qm4[:, c]
    return bd.astype(np.float16)


def pack_weights(weights, npc=NPC, n_cores=N_CORES):
    """Pack (N, 20) f32 weights into per-core fp16 strip streams.

    Per core: w_pk[r, (g*SLABS + B)*128 + p] = W[base_g + 672p + 6B + nu, k]
    with r = 20*nu + k.
    """
    wb16 = np.ascontiguousarray(weights).astype(np.float16)
    bases = _window_bases(npc)
    out = []
    for c in range(n_cores):
        wc = wb16[c * npc : (c + 1) * npc]
        blocks = []
        for b0 in bases:
            blk = wc[b0 : b0 + GWIN].reshape(128, SLABS, G_NODES, KW)
            # [p, B, nu, k] -> [nu, k, B, p] -> (120, SLABS*128)
            blocks.append(
                np.ascontiguousarray(blk.transpose(2, 3, 1, 0)).reshape(
                    F, SLABS * 128
                )
            )
        out.append(np.ascontiguousarray(np.concatenate(blocks, axis=1)))
    return out


_prog_cache = {}
_pack_cache = {}


def _get_program(npc, repeats=1, hw_loop=False, mode="full"):
    key = (npc, repeats, hw_loop, mode)
    if key not in _prog_cache:
        _prog_cache[key] = build_program(npc, repeats, hw_loop, mode=mode)
    return _prog_cache[key]


def _get_packed(weights, npc, n_cores):
    key = (id(weights), weights.ctypes.data, weights.shape)
    if key not in _pack_cache:
        _pack_cache.clear()
        _pack_cache[key] = pack_weights(weights, npc, n_cores)
    return _pack_cache[key]


def run(x, weights, VR, npc=NPC, n_cores=N_CORES, trace=False, repeats=1,
        hw_loop=False, mode="full"):
    weights = np.ascontiguousarray(np.asarray(weights, np.float32))
    VR = np.ascontiguousarray(np.asarray(VR, np.float32))
    bd = make_bd(x)
    w_pk = _get_packed(weights, npc, n_cores)
    nc = _get_program(npc, repeats, hw_loop, mode)
    in_maps = []
    for i in range(n_cores):
        in_maps.append(
            {
                "w": w_pk[i],
                "vr": VR[i * npc * 4 : (i + 1) * npc * 4],
                "bd": bd,
            }
        )
    res = run_bass_kernel_spmd(nc, in_maps, list(range(n_cores)), trace=trace)
    out = np.concatenate([res.results[i]["out"] for i in range(n_cores)])
    return out.astype(np.float32, copy=False), res


def kernel(x, weights, VR):
    out, _ = run(x, weights, VR)
    return out


# revision 10
# speedup vs baseline: 1.0419x; 1.0419x over previous
"""DQS encoder (dual-quaternion skinning blend) Trainium2 kernel, v2.

Contract: kernel(x, weights, VR) -> (8_000_000,) float32, matching
reference._dqs numerics. Data-parallel over nodes across 8 NeuronCores.

v2 strategy (vs the v1 strip-transpose pipeline): move the weight
transpose to the HOST and make the super-window (86,016 nodes) the
partition-chunking unit. Per core the 250k nodes are covered by 3
super-windows (last one overlaps, idempotent writes). Within a
super-window, partition p owns nodes [base + 672p, base + 672p + 672),
as 112 slabs of 6 nodes.

Host packs weights into a fp16 strip stream
  w_pk[r, (g*112 + B)*128 + p] = W[base_g + 672p + 6B + nu, k],
r = 20*nu + k, so every kernel DMA is fully contiguous per partition:

  1. W DMA per super-window: [120, 14336] fp16, 120 x 28KB descriptors.
  2. Per slab B: one matmul, stationary = strip slab [120, 128]
     (cols = p), moving = host-built block-diag bd [120, 32]:
     out[p, 6c+nu] = q_c(node 672p + 6B + nu). 16 slabs fill one PSUM
     bank [128, 512]; 7 banks' worth per super-window.
  3. One ACT copy PSUM -> t_sb per 16-slab chunk.
  4. VR DMA per super-window: [128, 2688] f32 contiguous
     (vrt[p, 4d + comp] = VR comp of node base + 672p + d, d = 6B + nu
     -- exactly matching the q layout t_sb[p, 32B + 6c + nu]).
  5. DVE math, cross-product form of the rotation (identical algebra
     to reference's G @ (H^T v), division by |q|^2 instead of
     normalize):
       t  = u x v + d*v
       y  = v + (1/|q|^2) * 2*(u x t)
     y written in-place over v slots of the VR tile; radius untouched.
  6. One contiguous DMA of the VR tile back out.

fp16 weights halve W HBM traffic (10MB/core/repeat); VR/out stay f32.
Per-core HBM traffic per repeat ~18MB -> ~50us roofline at 358GB/s.

Walrus's codegen accepts only ONE sync-wait command on most encodings
(incl. loop-barrier NoOps and Drains). Countermeasures: all SWDGE
completion semaphores are collapsed to one lane, and traced nop
placeholders are inserted before phase-leader instructions; a
post-schedule pass moves any excess waits onto the placeholders
(same-engine program order then enforces the dependency).

repeats can run as a hardware For_i loop (hw_loop=True): constant
program size for any repeat count, enabling honest repeat-slope timing.
"""

import sys
from contextlib import ExitStack

import numpy as np

sys.path.insert(0, "/opt/trn_rl_repo")

import concourse.bass as bass  # noqa: E402
import concourse.tile as tile  # noqa: E402
from concourse import mybir  # noqa: E402
from concourse import tile_sem_assignment as _tsa  # noqa: E402
from concourse.bass_utils import run_bass_kernel_spmd  # noqa: E402

# One SWDGE completion lane: DMA ticks subsume each other (the SWDGE queue
# is FIFO), so no instruction ever needs two DMA waits.
_tsa.NUM_SWDGE_GLOBAL_SEMS = 1

FP = mybir.dt.float32
BF = mybir.dt.float16
OP = mybir.AluOpType

N_NODES = 2_000_000
N_CORES = 8
NPC = N_NODES // N_CORES  # 250_000 nodes per core

G_NODES = 6          # nodes per strip column
KW = 20              # weights per node
F = G_NODES * KW     # 120 strip rows
CHUNK = 16           # slabs per PSUM bank fill (16 * 32 cols = 512)
N_CHUNK = 7          # PSUM-bank fills per super-window
SLABS = CHUNK * N_CHUNK          # 112 slabs per super-window
NPP = G_NODES * SLABS            # 672 nodes per partition
GWIN = 128 * NPP                 # 86_016 nodes per super-window


def _window_bases(npc):
    n_full = npc // GWIN
    bases = [w * GWIN for w in range(n_full)]
    if npc - n_full * GWIN > 0:
        bases.append(npc - GWIN)  # overlapped tail window (idempotent writes)
    return bases


def _fview(ap, off, dims):
    """Strided free-dim view of a 2-D SBUF/PSUM AP. dims = [[step, count],...]."""
    return bass.AP(tensor=ap.tensor, offset=ap.offset + off, ap=[ap.ap[0]] + dims)


class _Ph:
    """Pool of traced carrier nops for the excess-wait retarget pass."""

    def __init__(self):
        self.names = set()

    def seed(self, nc, pool, n_per_engine=96):
        sem = nc.alloc_semaphore("ph_carrier_dummy")
        for ns in (nc.tensor, nc.gpsimd, nc.scalar, nc.sync):
            for _ in range(n_per_engine):
                p = ns.wait_ge(sem, 0).ins
                self.names.add(p.name)
        junk = pool.tile([1, n_per_engine], mybir.dt.float32, tag="phjunk")
        for k in range(n_per_engine):
            p = nc.vector.memset(junk[0:1, k : k + 1], 0.0).ins
            self.names.add(p.name)


def _retarget_waits(nc, ph_names):
    """Relocate seeded carrier nops to absorb excess sync-waits (walrus
    accepts only one sync-wait on most instruction encodings)."""
    import bass_rust

    moved = 0
    skip = ("InstEventSemaphore",)
    allow = ("InstMatmult", "InstActivation", "InstDMACopy", "InstDrain",
             "InstTensorTensor", "InstTensorScalarPtr", "InstTensorReduce",
             "InstReciprocal", "InstCopy", "InstTensorCopy", "InstNoOp",
             "InstMemset")
    blocks = list(nc.main_func.blocks)
    pool = {}
    plan = {}
    consumed = set()
    for bb in blocks:
        for ins in bb.instructions:
            if ins.name in ph_names and (
                ins.sync_info is None or not ins.sync_info.on_update
            ):
                pool.setdefault(ins.engine, []).append(ins)
    for bb in blocks:
        for ins in bb.instructions:
            if ins.name in ph_names:
                continue
            if type(ins).__name__ in skip or type(ins).__name__ not in allow:
                continue
            si = ins.sync_info
            if si is not None and len(si.on_wait) > 1:
                waits = list(si.on_wait)
                excess = waits[:-1]
                phs = pool.get(ins.engine, [])
                carriers = []
                for w in excess:
                    if phs:
                        p = phs.pop()
                    else:
                        # Synthesize a fresh carrier nop (no seeded carrier
                        # left on this engine, e.g. DVE where every traced
                        # op acquires a clock update).
                        p = mybir.InstNoOp(
                            name=nc.get_next_instruction_name(),
                            engine=ins.engine, ins=[], outs=[],
                        )
                        ph_names.add(p.name)
                    p.sync_info = bass_rust.SyncInfo(on_wait=[w], on_update=[])
                    try:
                        p.bass_scheduled_tick = ins.bass_scheduled_tick
                    except Exception:
                        pass
                    consumed.add(p.name)
                    carriers.append(p)
                    moved += 1
                ins.sync_info = bass_rust.SyncInfo(
                    on_wait=waits[-1:], on_update=list(si.on_update)
                )
                plan[ins.name] = carriers
    unused = set()
    for phs in pool.values():
        unused.update(p.name for p in phs)
    for bb in blocks:
        out = []
        for ins in bb.instructions:
            if ins.name in consumed or ins.name in unused:
                continue
            out.extend(plan.get(ins.name, ()))
            out.append(ins)
        bb.instructions = out
    return moved


def _fix_loop_swdge_reset(nc):
    """Walrus's codegen cannot encode InstIncSwdgeSem ("ISA wrong
    length"), which Tile's For_i reset block uses to rewind the SWDGE
    completion semaphore between iterations. Replace it with a Pool NoOp
    carrying a sem-sub-imm update of the per-iteration SWDGE total (the
    same mechanism Tile uses for the engine-clock resets). All body DMAs
    are complete at the reset point (the reset block is behind a drain +
    all-engine barrier), so the value is statically exact."""
    import bass_rust

    blocks = list(nc.main_func.blocks)
    body = [bb for bb in blocks if bb.name.endswith("_body")]
    if not body:
        return 0
    # per-sem SWDGE increment totals inside the loop body
    totals = {}
    names = {}
    for bb in body:
        for ins in bb.instructions:
            si = ins.sync_info
            if type(ins).__name__ == "InstDMACopy" and si is not None:
                for u in si.on_update:
                    if u.ant_name.startswith("DMASW"):
                        totals[u.id] = totals.get(u.id, 0) + u.update_value
                        names[u.id] = u.ant_name
    replaced = 0
    for bb in blocks:
        out = []
        for ins in bb.instructions:
            if type(ins).__name__ != "InstIncSwdgeSem":
                out.append(ins)
                continue
            si = ins.sync_info
            waits = list(si.on_wait) if si else []
            if bb.name.endswith("_reset"):
                ups = [
                    bass_rust.SyncUpdate(
                        sync_type="semaphore", id=i, ant_name=names[i],
                        update_mode="sem-sub-imm", update_value=v,
                        update_reg=None,
                    )
                    for i, v in sorted(totals.items()) if v > 0
                ]
            else:
                # skip-block instance: loop trip count is always >= 2 in
                # hw_loop mode, the skip path never executes; keep only
                # its wait so the block stays well-formed.
                ups = []
            rep = mybir.InstNoOp(
                name=nc.get_next_instruction_name(),
                engine=ins.engine, ins=[], outs=[],
            )
            rep.sync_info = bass_rust.SyncInfo(
                on_wait=waits[:1], on_update=ups[:1]
            )
            out.append(rep)
            for extra in ups[1:]:
                rep2 = mybir.InstNoOp(
                    name=nc.get_next_instruction_name(),
                    engine=ins.engine, ins=[], outs=[],
                )
                rep2.sync_info = bass_rust.SyncInfo(on_wait=[], on_update=[extra])
                out.append(rep2)
            replaced += 1
        bb.instructions = out
    return replaced


def build_program(npc=NPC, repeats=1, hw_loop=False, split_waits=True,
                  mode="full"):
    # mode: "full" | "nomath" (skip DVE math; out = raw VR) |
    #       "dma" (only the three DMAs) — timing ablations, output wrong.
    nc = bass.Bass()

    bases = _window_bases(npc)
    nwin = len(bases)

    w_d = nc.dram_tensor("w", [F, nwin * SLABS * 128], BF, kind="ExternalInput")
    vr_d = nc.dram_tensor("vr", [npc * 4], FP, kind="ExternalInput")
    bd_d = nc.dram_tensor("bd", [F, 32], BF, kind="ExternalInput")
    out_d = nc.dram_tensor("out", [npc * 4], FP, kind="ExternalOutput")

    unroll = 1 if hw_loop else repeats

    ph = _Ph()

    with tile.TileContext(nc) as tc, ExitStack() as ctx:
        const = ctx.enter_context(tc.tile_pool(name="const", bufs=1))
        ph.seed(nc, const, n_per_engine=64 + 48 * unroll)
        strip_p = ctx.enter_context(tc.tile_pool(name="strip", bufs=2))
        grp_p = ctx.enter_context(tc.tile_pool(name="grp", bufs=2))
        scr_p = ctx.enter_context(tc.tile_pool(name="scr", bufs=1))
        sq_p = ctx.enter_context(tc.tile_pool(name="sq", bufs=2))
        qps_p = ctx.enter_context(tc.tile_pool(name="qps", bufs=4, space="PSUM"))

        bd_sb = const.tile([F, 32], BF)
        nc.sync.dma_start(out=bd_sb[:], in_=bd_d[:, :])
        s0_junk = const.tile([1, 16], FP)

        if hw_loop and repeats > 1:
            # Hardware repeat loop: constant program size for any repeat
            # count (same addresses each iteration, idempotent writes).
            ctx.enter_context(tc.For_i(0, repeats, 1))

        for _rep in range(unroll):
            for gw, base in enumerate(bases):
                # --- W strip load: one DMA per 16-slab chunk (sync HWDGE
                # ring) so blend matmuls start as soon as their chunk
                # lands instead of waiting for the whole 3.4MB window ---
                strip = strip_p.tile([F, SLABS * 128], BF, tag="strip")
                cw = CHUNK * 128
                for ch in range(N_CHUNK):
                    nc.sync.dma_start(
                        out=strip[:, ch * cw : (ch + 1) * cw],
                        in_=w_d[:, gw * SLABS * 128 + ch * cw :
                                gw * SLABS * 128 + (ch + 1) * cw],
                    )

                # --- VR load: one contiguous DMA per super-window, on the
                # scalar HWDGE ring so it doesn't queue behind W ---
                vrt = grp_p.tile([128, NPP * 4], FP, tag="vrt")
                nc.scalar.dma_start(
                    out=vrt[:],
                    in_=vr_d[base * 4 : (base + GWIN) * 4].rearrange(
                        "(p f) -> p f", p=128
                    ),
                )

                if mode == "dma":
                    r0 = vrt[0:1, 3:4]
                    nc.vector.tensor_copy(r0, r0)  # WAR split vs prev out-DMA
                    nc.sync.dma_start(
                        out=out_d[base * 4 : (base + GWIN) * 4].rearrange(
                            "(p f) -> p f", p=128
                        ),
                        in_=vrt[:],
                    )
                    # keep the strip load live (read 1 elem so DCE keeps it)
                    nc.vector.tensor_copy(s0_junk[0:1, gw : gw + 1],
                                          strip[0:1, 0:1])
                    continue

                # --- blend matmuls + PSUM drain ---
                t_sb = grp_p.tile([128, N_CHUNK * 512], FP, tag="t_sb")
                for ch in range(N_CHUNK):
                    qps = qps_p.tile([128, 512], FP, tag="qps")
                    for b in range(CHUNK):
                        col = (ch * CHUNK + b) * 128
                        nc.tensor.matmul(
                            qps[:, 32 * b : 32 * (b + 1)],
                            strip[:, col : col + 128],
                            bd_sb[:],
                            start=True, stop=True, tile_position=(0, 0),
                        )
                    nc.scalar.copy(t_sb[:, 512 * ch : 512 * (ch + 1)], qps[:])

                if mode == "nomath":
                    nc.vector.tensor_copy(s0_junk[0:1, gw : gw + 1],
                                          t_sb[0:1, 0:1])
                    r0 = vrt[0:1, 3:4]
                    nc.vector.tensor_copy(r0, r0)
                    nc.sync.dma_start(
                        out=out_d[base * 4 : (base + GWIN) * 4].rearrange(
                            "(p f) -> p f", p=128
                        ),
                        in_=vrt[:],
                    )
                    continue

                # ---------- math over the super-window ----------
                a = SLABS       # merged slab dim
                fd = 6 * a      # nodes per partition (672)

                def qv(c):  # quat component plane view of t_sb
                    return _fview(t_sb[:], 6 * c, [[32, a], [1, 6]])

                def vv(c):  # VR component plane view
                    return _fview(vrt[:], c, [[24, a], [4, 6]])

                def sh(tl):  # scratch tile shaped to match views
                    return _fview(tl[:], 0, [[6, a], [1, 6]])

                A, Bq, C, D = qv(0), qv(1), qv(2), qv(3)
                v1, v2, v3 = vv(0), vv(1), vv(2)

                t1 = scr_p.tile([128, fd], FP, tag="t1")
                t2 = scr_p.tile([128, fd], FP, tag="t2")
                t3 = scr_p.tile([128, fd], FP, tag="t3")
                s1 = scr_p.tile([128, fd], FP, tag="s1")
                s2 = scr_p.tile([128, fd], FP, tag="s2")

                # Split the multi-producer dependency join of the first DVE
                # op across chained 1-element copies (walrus: one wait/op).
                # Dest s2 is clobbered later.
                for ch in range(N_CHUNK):
                    nc.vector.tensor_copy(
                        s2[0:1, ch : ch + 1],
                        t_sb[0:1, 512 * ch : 512 * ch + 1],
                    )
                nc.vector.tensor_copy(s2[0:1, 7:8], vrt[0:1, 0:1])

                def tt(out, a_, b_, op):
                    nc.vector.tensor_tensor(out, a_, b_, op)

                def stt(out, in0, scalar, in1):
                    nc.vector.scalar_tensor_tensor(
                        out=out, in0=in0, scalar=scalar, in1=in1,
                        op0=OP.mult, op1=OP.mult)

                w1 = scr_p.tile([128, fd], FP, tag="w1")
                w2 = scr_p.tile([128, fd], FP, tag="w2")
                w3 = scr_p.tile([128, fd], FP, tag="w3")
                n2 = scr_p.tile([128, fd], FP, tag="n2")
                inv = scr_p.tile([128, fd], FP, tag="inv")
                sq = sq_p.tile([128, 24 * a], FP, tag="sq")

                # t = u x v + d*v
                for tout, (f1, e1), (f2, e2), (f3, e3) in (
                    (t1, (Bq, v3), (C, v2), (D, v1)),
                    (t2, (C, v1), (A, v3), (D, v2)),
                    (t3, (A, v2), (Bq, v1), (D, v3)),
                ):
                    tt(sh(s1), f1, e1, OP.mult)
                    stt(sh(s2), f2, -1.0, e2)
                    tt(sh(s1), sh(s1), sh(s2), OP.add)
                    tt(sh(s2), f3, e3, OP.mult)
                    tt(sh(tout), sh(s1), sh(s2), OP.add)

                # w = 2*(u x t)
                for wout, (f1, e1), (f2, e2) in (
                    (w1, (Bq, t3), (C, t2)),
                    (w2, (C, t1), (A, t3)),
                    (w3, (A, t2), (Bq, t1)),
                ):
                    stt(sh(s1), f1, 2.0, sh(e1))
                    stt(sh(s2), f2, 2.0, sh(e2))
                    tt(sh(wout), sh(s1), sh(s2), OP.subtract)

                # n2 = sum of squares over the 4 quat components; inv = 1/n2
                sq_in = _fview(t_sb[:], 0, [[32, a], [1, 24]])
                sq_out = _fview(sq[:], 0, [[24, a], [1, 24]])
                nc.scalar.activation(
                    sq_out, sq_in, mybir.ActivationFunctionType.Square,
                )
                sqv = _fview(sq[:], 0, [[24, a], [1, 6], [6, 4]])
                nc.vector.tensor_reduce(
                    out=sh(n2), in_=sqv, axis=mybir.AxisListType.X, op=OP.add
                )
                nc.vector.reciprocal(out=inv[:, :fd], in_=n2[:, :fd])

                # WAR split: the y-passes write vrt in place and would
                # inherit waits on the previous occupant's out-DMA.
                r0 = vrt[0:1, 3:4]
                nc.vector.tensor_copy(r0, r0)

                # y_c = v_c + inv * w_c   (written in place over v_c)
                for wsrc, vdst_ in ((w1, v1), (w2, v2), (w3, v3)):
                    tt(sh(s1), sh(inv), sh(wsrc), OP.mult)
                    tt(vdst_, sh(s1), vdst_, OP.add)

                # --- store: one contiguous DMA per super-window (scalar
                # ring, parallel to the next window's W loads) ---
                nc.scalar.dma_start(
                    out=out_d[base * 4 : (base + GWIN) * 4].rearrange(
                        "(p f) -> p f", p=128
                    ),
                    in_=vrt[:],
                )

    if split_waits:
        _retarget_waits(nc, ph.names)
    if hw_loop and repeats > 1:
        _fix_loop_swdge_reset(nc)
    return nc


def make_bd(x):
    """Host-side block-diag blend matrix (120, 32) bf16 from x (40,)."""
    qm4p1 = np.asarray(x, np.float32).reshape(10, 4)
    qm4p2 = np.zeros_like(qm4p1)
    qm4p2[:, 3] = 1.0
    qm4 = np.concatenate([qm4p1, qm4p2], axis=0)  # (20, 4)
    bd = np.zeros((F, 32), np.float32)
    for nu in range(G_NODES):
        for c in range(4):
            bd[KW * nu : KW * (nu + 1), 6 * c + nu] = qm4[:, c]
    return bd.astype(np.float16)


def pack_weights(weights, npc=NPC, n_cores=N_CORES):
    """Pack (N, 20) f32 weights into per-core fp16 strip streams.

    Per core: w_pk[r, (g*SLABS + B)*128 + p] = W[base_g + 672p + 6B + nu, k]
    with r = 20*nu + k.
    """
    wb16 = np.ascontiguousarray(weights).astype(np.float16)
    bases = _window_bases(npc)
    out = []
    for c in range(n_cores):
        wc = wb16[c * npc : (c + 1) * npc]
        blocks = []
        for b0 in bases:
            blk = wc[b0 : b0 + GWIN].reshape(128, SLABS, G_NODES, KW)
            # [p, B, nu, k] -> [nu, k, B, p] -> (120, SLABS*128)
            blocks.append(
                np.ascontiguousarray(blk.transpose(2, 3, 1, 0)).reshape(
                    F, SLABS * 128
                )
            )
        out.append(np.ascontiguousarray(np.concatenate(blocks, axis=1)))
    return out


_prog_cache = {}
_pack_cache = {}


def _get_program(npc, repeats=1, hw_loop=False, mode="full"):
    key = (npc, repeats, hw_loop, mode)
    if key not in _prog_cache:
        _prog_cache[key] = build_program(npc, repeats, hw_loop, mode=mode)
    return _prog_cache[key]


def _get_packed(weights, npc, n_cores):
    key = (id(weights), weights.ctypes.data, weights.shape)
    if key not in _pack_cache:
        _pack_cache.clear()
        _pack_cache[key] = pack_weights(weights, npc, n_cores)
    return _pack_cache[key]


def run(x, weights, VR, npc=NPC, n_cores=N_CORES, trace=False, repeats=1,
        hw_loop=False, mode="full"):
    weights = np.ascontiguousarray(np.asarray(weights, np.float32))
    VR = np.ascontiguousarray(np.asarray(VR, np.float32))
    bd = make_bd(x)
    w_pk = _get_packed(weights, npc, n_cores)
    nc = _get_program(npc, repeats, hw_loop, mode)
    in_maps = []
    for i in range(n_cores):
        in_maps.append(
            {
                "w": w_pk[i],
                "vr": VR[i * npc * 4 : (i + 1) * npc * 4],
                "bd": bd,
            }
        )
    res = run_bass_kernel_spmd(nc, in_maps, list(range(n_cores)), trace=trace)
    out = np.concatenate([res.results[i]["out"] for i in range(n_cores)])
    return out.astype(np.float32, copy=False), res


def kernel(x, weights, VR):
    out, _ = run(x, weights, VR)
    return out
